# revision 27
# baseline (speedup 1.0000x reference)
"""AFT-Local distributed Trainium2 kernel (8 NeuronCores).

Math (reference, with cancellations):
  q = query @ Wq.T; k = key_in @ Wk.T; v = value @ Wv.T      [S,B,D]
  E[i,j] = exp(pos_bias[i,j] * (j <= i-255))                 [S,S]
  num[i,b,:] = sum_j E[i,j] * (exp(k)*v)[j,b,:]
  den[i,b,:] = sum_j E[i,j] *  exp(k)[j,b,:]
  out = (sigmoid(q) * num / den) @ Wo.T
The max-subtractions in the reference cancel in num/den; all values are small
enough that plain exp is safe.

Distribution (v4+): pure data/tensor-parallel, ZERO device collectives (the
collective control path on this fleet has a ~90us fixed cost, impossible to
hide). Core c owns (batch b = c//2, d-half h = c%2): it projects k/v/q for
all 2048 tokens restricted to its 512 d-columns (no duplicated FLOPs), runs
the full [2048x2048] E-weighted attention on its slice entirely out of SBUF,
and computes a PARTIAL output projection over its d-half. The host sums each
core-pair's f32 partials while unsharding - the only cross-core data motion
in the whole scheme.

Kernel structure (v5): all matmuls bf16 with f32 PSUM accumulation, in long
accumulation chains into a single PSUM bank (keeps the PE HAM-warm). The
attention num/den and the q projection run in the TRANSPOSED [d,i]
orientation so y comes out as y^T and feeds the output projection directly -
no on-chip transposes anywhere. The local mask is pre-applied to pos_bias^T
on the host (static index mask) so the device only exponentiates.
"""

import os
import sys

import numpy as np
import ml_dtypes

sys.path.insert(0, "/opt/trn_rl_repo")

S, B, D, W = 2048, 4, 1024, 256
NC = 8
P = 128
NT = S // P  # 16 token/row tiles
DH = 512  # d-half owned per core

_CACHE = {}


def _build():
    import concourse.bass as bass
    import concourse.bacc as bacc
    import concourse.mybir as mybir
    import concourse.tile as tile

    f32 = mybir.dt.float32
    bf16 = mybir.dt.bfloat16
    AF = mybir.ActivationFunctionType

    nc = bacc.Bacc("TRN2", target_bir_lowering=False, debug=False, num_devices=NC)

    # per-core inputs (b = batch owned, h = d-half owned)
    keyT = nc.dram_tensor("keyT", [D, S], bf16, kind="ExternalInput")  # key_in[:,b,:].T
    valT = nc.dram_tensor("valT", [D, S], bf16, kind="ExternalInput")
    queryT = nc.dram_tensor("queryT", [D, S], bf16, kind="ExternalInput")
    pbT = nc.dram_tensor("pbT", [S, S], bf16, kind="ExternalInput")  # masked pos_bias^T
    wk = nc.dram_tensor("wk", [D, DH], bf16, kind="ExternalInput")  # Wk.T[:, h-cols]
    wv = nc.dram_tensor("wv", [D, DH], bf16, kind="ExternalInput")
    wq = nc.dram_tensor("wq", [D, DH], bf16, kind="ExternalInput")
    wo = nc.dram_tensor("wo", [DH, D], bf16, kind="ExternalInput")  # Wo.T[h-rows, :]
    out = nc.dram_tensor("out", [S, D], f32, kind="ExternalOutput")  # partial!

    with tile.TileContext(nc) as tc:
        with tc.tile_pool(name="persist", bufs=1) as persist:
            # resident across phases (per-partition KB in comments)
            ek_sb = [persist.tile([P, DH], bf16, name=f"ek{t}") for t in range(NT)]    # 16
            ekv_sb = [persist.tile([P, DH], bf16, name=f"ekv{t}") for t in range(NT)]  # 16
            eT_sb = [persist.tile([P, S], bf16, name=f"eT{t}") for t in range(NT - 1)] # 60
            sqT_sb = [persist.tile([P, S], bf16, name=f"sqT{t}") for t in range(4)]    # 16
            yT_sb = [persist.tile([P, S], bf16, name=f"yT{t}") for t in range(4)]      # 16
            ones_j = persist.tile([P, 1], bf16, name="ones_j")
            ones_row = persist.tile([1, 512], bf16, name="ones_row")
            stot_sb = persist.tile([1, 512], bf16, name="stot_sb")
            ktot_sb = persist.tile([1, 512], bf16, name="ktot_sb")
            nc.gpsimd.memset(ones_j[:], 1.0)
            nc.gpsimd.memset(ones_row[:], 1.0)
            # j-tile reach per 512-row i-block: j <= i-255
            CAPS = [2, 6, 10, 14]
            IB_MIN = [0 if j <= 2 else 1 if j <= 6 else 2 if j <= 10 else 3
                      for j in range(NT - 1)]

            # ---- phase A: k/v projection (all tokens, own d-half), exp ----
            # two token-halves so keyT/valT are only half-resident
            with (
                tc.tile_pool(name="pa", bufs=1) as pa,
                tc.tile_pool(name="pa_st", bufs=3) as pa_st,
                tc.tile_pool(name="ps_a", bufs=2, space="PSUM") as ps_a,
            ):
                pd = tc.alloc_tile_pool(name="pd", bufs=3)
                # weights as one [128, 8*512] tile: block kt at cols kt*512
                wk_sb = pa.tile([P, 8 * DH], bf16, name="wk_sb")
                nc.sync.dma_start(
                    out=wk_sb[:], in_=wk[:, :].rearrange("(kt p) e -> p kt e", p=P)
                )
                wv_sb = pa.tile([P, 8 * DH], bf16, name="wv_sb")
                nc.sync.dma_start(
                    out=wv_sb[:], in_=wv[:, :].rearrange("(kt p) e -> p kt e", p=P)
                )
                # token quarters, double-buffered so loads prefetch ahead of
                # the WAR release; phase-D tiles interleave into the DMA gaps
                for q in range(4):
                    cs = slice(q * 512, (q + 1) * 512)
                    keyT_sb = pa.tile(
                        [P, 8 * 512], bf16, tag="keyT_q", name="keyT_q", bufs=2
                    )
                    nc.sync.dma_start(
                        out=keyT_sb[:],
                        in_=keyT[:, :].rearrange("(kt p) s -> p kt s", p=P)[:, :, cs],
                    )
                    valT_sb = pa.tile(
                        [P, 8 * 512], bf16, tag="valT_q", name="valT_q", bufs=2
                    )
                    nc.sync.dma_start(
                        out=valT_sb[:],
                        in_=valT[:, :].rearrange("(kt p) s -> p kt s", p=P)[:, :, cs],
                    )
                    for jt in range(4 * q, min(4 * q + 4, NT - 1)):
                        # only the unmasked column range is ever read
                        c0 = 512 * IB_MIN[jt]
                        pbt = pd.tile([P, S], bf16, tag="pbt")
                        nc.sync.dma_start(
                            out=pbt[:, c0:S], in_=pbT[jt * P : (jt + 1) * P, c0:S]
                        )
                        nc.scalar.activation(eT_sb[jt][:, c0:S], pbt[:, c0:S], AF.Exp)
                        nc.vector.tensor_scalar_add(
                            eT_sb[jt][:, c0:S], eT_sb[jt][:, c0:S], -1.0
                        )
                    for tl in range(4):
                        tt = q * 4 + tl
                        psk = ps_a.tile([P, DH], f32, tag="psk")
                        psv = ps_a.tile([P, DH], f32, tag="psv")
                        for kt in range(8):
                            c = kt * 512 + tl * P
                            nc.tensor.matmul(
                                psk[:],
                                keyT_sb[:, c : c + P],
                                wk_sb[:, kt * DH : (kt + 1) * DH],
                                start=(kt == 0),
                                stop=(kt == 7),
                            )
                        for kt in range(8):
                            c = kt * 512 + tl * P
                            nc.tensor.matmul(
                                psv[:],
                                valT_sb[:, c : c + P],
                                wv_sb[:, kt * DH : (kt + 1) * DH],
                                start=(kt == 0),
                                stop=(kt == 7),
                            )
                        ekf = pa_st.tile([P, DH], f32, tag="ekf")
                        nc.scalar.activation(ekf[:], psk[:], AF.Exp)
                        nc.vector.tensor_copy(ek_sb[tt][:], ekf[:])
                        nc.vector.tensor_mul(ekv_sb[tt][:], ekf[:], psv[:])
                pd.release()

            # ---- phase C: q^T projection + sigmoid ([e,i] orientation) ----
            with (
                tc.tile_pool(name="pc", bufs=1) as pc,
                tc.tile_pool(name="ps_c", bufs=2, space="PSUM") as ps_c,
            ):
                wq_sb = pc.tile([P, 8 * DH], bf16, name="wq_sb")
                nc.sync.dma_start(
                    out=wq_sb[:], in_=wq[:, :].rearrange("(kt p) e -> p kt e", p=P)
                )
                for ib in range(4):
                    cs = slice(ib * 512, (ib + 1) * 512)
                    qT_sb = pc.tile(
                        [P, 8 * 512], bf16, tag="qT_q", name="qT_q", bufs=2
                    )
                    nc.sync.dma_start(
                        out=qT_sb[:],
                        in_=queryT[:, :].rearrange("(kt p) s -> p kt s", p=P)[:, :, cs],
                    )
                    for et in range(4):
                        psq = ps_c.tile([P, 512], f32, tag="psq")
                        for kt in range(8):
                            nc.tensor.matmul(
                                psq[:],
                                wq_sb[:, kt * DH + et * P : kt * DH + (et + 1) * P],
                                qT_sb[:, kt * 512 : (kt + 1) * 512],
                                start=(kt == 0),
                                stop=(kt == 7),
                            )
                        nc.scalar.activation(
                            sqT_sb[et][:, ib * 512 : (ib + 1) * 512], psq[:], AF.Sigmoid
                        )
                # Stot/Ktot: token-tile accumulation on the idle GpSimd,
                # then one M=1 matmul each for the partition reduction
                ps_s = tc.alloc_tile_pool(name="ps_s", bufs=1, space="PSUM")
                pacc = tc.alloc_tile_pool(name="pacc", bufs=1)
                sacc = pacc.tile([P, 512], f32, name="sacc")
                kacc = pacc.tile([P, 512], f32, name="kacc")
                nc.gpsimd.tensor_copy(sacc[:], ekv_sb[0][:])
                nc.gpsimd.tensor_copy(kacc[:], ek_sb[0][:])
                for jt in range(1, NT):
                    nc.gpsimd.tensor_add(sacc[:], sacc[:], ekv_sb[jt][:])
                    nc.gpsimd.tensor_add(kacc[:], kacc[:], ek_sb[jt][:])
                stp = ps_s.tile([1, 512], f32, tag="stp")
                ktp = ps_s.tile([1, 512], f32, tag="ktp")
                onesf = pacc.tile([P, 1], f32, name="onesf")
                nc.vector.memset(onesf[:], 1.0)
                nc.tensor.matmul(stp[:], onesf[:], sacc[:], start=True, stop=True)
                nc.tensor.matmul(ktp[:], onesf[:], kacc[:], start=True, stop=True)
                nc.vector.tensor_copy(stot_sb[:], stp[:])
                nc.vector.tensor_copy(ktot_sb[:], ktp[:])
                ps_s.release()
                pacc.release()

            # ---- phase E: num^T/den^T chains in [d,i]; y^T epilogue ----
            # descending cap order: long chains first (PE stays warm) and the
            # late i-blocks' y^T land early so phase F can overlap the tail
            with (
                tc.tile_pool(name="pe_ep", bufs=2) as pe_ep,
                tc.tile_pool(name="ps_e", bufs=2, space="PSUM") as ps_e,
            ):
                for ib in (3, 2, 1, 0):
                    csl = slice(ib * 512, (ib + 1) * 512)
                    cap = CAPS[ib]
                    for dt in range(4):
                        dsl = slice(dt * P, (dt + 1) * P)
                        na = ps_e.tile([P, 512], f32, tag="na")
                        da = ps_e.tile([P, 512], f32, tag="da")
                        # rank-1 dense term (all-j sum), then the expm1
                        # triangular correction over the reachable j-tiles
                        nc.tensor.matmul(
                            na[:], stot_sb[0:1, dt * P : (dt + 1) * P],
                            ones_row[:], start=True, stop=False,
                        )
                        for jt in range(cap + 1):
                            nc.tensor.matmul(
                                na[:],
                                ekv_sb[jt][:, dsl],
                                eT_sb[jt][:, csl],
                                start=False,
                                stop=(jt == cap),
                            )
                        nc.tensor.matmul(
                            da[:], ktot_sb[0:1, dt * P : (dt + 1) * P],
                            ones_row[:], start=True, stop=False,
                        )
                        for jt in range(cap + 1):
                            nc.tensor.matmul(
                                da[:],
                                ek_sb[jt][:, dsl],
                                eT_sb[jt][:, csl],
                                start=False,
                                stop=(jt == cap),
                            )
                        rec = pe_ep.tile([P, 512], f32, tag="rec")
                        nc.vector.reciprocal(rec[:], da[:])
                        tmp = pe_ep.tile([P, 512], f32, tag="tmp")
                        nc.vector.tensor_mul(tmp[:], na[:], rec[:])
                        # final gate-mul on the otherwise-idle GpSimd (SBUF-only)
                        nc.gpsimd.tensor_mul(
                            yT_sb[dt][:, csl], tmp[:], sqT_sb[dt][:, csl]
                        )

            # ---- phase F: partial output projection (y^T is the lhsT) ----
            with (
                tc.tile_pool(name="pf_o", bufs=3) as pf_o,
                tc.tile_pool(name="ps_fo", bufs=2, space="PSUM") as ps_fo,
            ):
                wo_sb = pf_o.tile([P, 4 * D], bf16, name="wo_sb", tag="wo_sb", bufs=1)
                nc.sync.dma_start(
                    out=wo_sb[:], in_=wo[:, :].rearrange("(dt p) e -> p dt e", p=P)
                )
                for it in range(NT - 1, -1, -1):
                    for es in range(2):
                        pso = ps_fo.tile([P, 512], f32, tag=f"pso{es}")
                        for dt in range(4):
                            nc.tensor.matmul(
                                pso[:],
                                yT_sb[dt][:, it * P : (it + 1) * P],
                                wo_sb[:, dt * D + es * 512 : dt * D + (es + 1) * 512],
                                start=(dt == 0),
                                stop=(dt == 3),
                            )
                        osb = pf_o.tile([P, 512], f32, tag="osb")
                        nc.scalar.activation(osb[:], pso[:], AF.Copy)
                        nc.sync.dma_start(
                            out=out[it * P : (it + 1) * P, es * 512 : (es + 1) * 512],
                            in_=osb[:],
                        )

    nc.compile()
    return nc


def _prep_inputs(inputs):
    bf = ml_dtypes.bfloat16
    query, key_in, value = inputs["query"], inputs["key_in"], inputs["value"]
    pos_bias = inputs["pos_bias"]

    jj = np.arange(S)
    pbT = pos_bias.T.copy()  # [j, i]
    pbT[~(jj[:, None] <= jj[None, :] - (W - 1))] = 0.0
    pbT = pbT.astype(bf)

    wq_t = np.ascontiguousarray(inputs["Wq"].T).astype(bf)  # [din, e]
    wk_t = np.ascontiguousarray(inputs["Wk"].T).astype(bf)
    wv_t = np.ascontiguousarray(inputs["Wv"].T).astype(bf)
    wo_t = np.ascontiguousarray(inputs["Wo"].T).astype(bf)  # [d, e']

    keyT_b = [np.ascontiguousarray(key_in[:, b, :].T).astype(bf) for b in range(B)]
    valT_b = [np.ascontiguousarray(value[:, b, :].T).astype(bf) for b in range(B)]
    qT_b = [np.ascontiguousarray(query[:, b, :].T).astype(bf) for b in range(B)]

    in_maps = []
    for c in range(NC):
        b, h = c // 2, c % 2
        hs = slice(h * DH, (h + 1) * DH)
        in_maps.append(
            {
                "keyT": keyT_b[b],
                "valT": valT_b[b],
                "queryT": qT_b[b],
                "pbT": pbT,
                "wk": np.ascontiguousarray(wk_t[:, hs]),
                "wv": np.ascontiguousarray(wv_t[:, hs]),
                "wq": np.ascontiguousarray(wq_t[:, hs]),
                "wo": np.ascontiguousarray(wo_t[hs, :]),
            }
        )
    return in_maps


def _run(inputs, trace=False):
    from concourse.bass_utils import run_bass_kernel_spmd

    if "nc" not in _CACHE:
        _CACHE["nc"] = _build()
    nc = _CACHE["nc"]

    in_maps = _prep_inputs(inputs)
    res = run_bass_kernel_spmd(nc, in_maps, core_ids=list(range(NC)), trace=trace)

    # unshard: partial sums over d-halves per batch
    full = np.empty((S, B, D), np.float32)
    for b in range(B):
        p0 = np.asarray(res.results[2 * b]["out"], np.float32)
        p1 = np.asarray(res.results[2 * b + 1]["out"], np.float32)
        full[:, b, :] = p0 + p1
    return full, res


def kernel(**inputs):
    inputs = {k: np.asarray(v) for k, v in inputs.items()}
    full, _ = _run(inputs, trace=False)
    return full


if __name__ == "__main__":
    inputs = np.load("/tmp/inputs.npy", allow_pickle=True).item()
    out = kernel(**inputs)
    print("out", out.shape, out.dtype)


# revision 28
# speedup vs baseline: 1.0982x; 1.0982x over previous
"""AFT-Local distributed Trainium2 kernel (8 NeuronCores).

Math (reference, with cancellations):
  q = query @ Wq.T; k = key_in @ Wk.T; v = value @ Wv.T      [S,B,D]
  E[i,j] = exp(pos_bias[i,j] * (j <= i-255))                 [S,S]
  num[i,b,:] = sum_j E[i,j] * (exp(k)*v)[j,b,:]
  den[i,b,:] = sum_j E[i,j] *  exp(k)[j,b,:]
  out = (sigmoid(q) * num / den) @ Wo.T
The max-subtractions in the reference cancel in num/den; all values are small
enough that plain exp is safe.

Distribution (v4+): pure data/tensor-parallel, ZERO device collectives (the
collective control path on this fleet has a ~90us fixed cost, impossible to
hide). Core c owns (batch b = c//2, d-half h = c%2): it projects k/v/q for
all 2048 tokens restricted to its 512 d-columns (no duplicated FLOPs), runs
the full [2048x2048] E-weighted attention on its slice entirely out of SBUF,
and computes a PARTIAL output projection over its d-half. The host sums each
core-pair's f32 partials while unsharding - the only cross-core data motion
in the whole scheme.

Kernel structure (v5): all matmuls bf16 with f32 PSUM accumulation, in long
accumulation chains into a single PSUM bank (keeps the PE HAM-warm). The
attention num/den and the q projection run in the TRANSPOSED [d,i]
orientation so y comes out as y^T and feeds the output projection directly -
no on-chip transposes anywhere. The local mask is pre-applied to pos_bias^T
on the host (static index mask) so the device only exponentiates.
"""

import os
import sys

import numpy as np
import ml_dtypes

sys.path.insert(0, "/opt/trn_rl_repo")

S, B, D, W = 2048, 4, 1024, 256
NC = 8
P = 128
NT = S // P  # 16 token/row tiles
DH = 512  # d-half owned per core

_CACHE = {}


def _build():
    import concourse.bass as bass
    import concourse.bacc as bacc
    import concourse.mybir as mybir
    import concourse.tile as tile

    f32 = mybir.dt.float32
    bf16 = mybir.dt.bfloat16
    AF = mybir.ActivationFunctionType

    nc = bacc.Bacc("TRN2", target_bir_lowering=False, debug=False, num_devices=NC)

    # per-core inputs (b = batch owned, h = d-half owned)
    keyT = nc.dram_tensor("keyT", [D, S], bf16, kind="ExternalInput")  # key_in[:,b,:].T
    valT = nc.dram_tensor("valT", [D, S], bf16, kind="ExternalInput")
    queryT = nc.dram_tensor("queryT", [D, S], bf16, kind="ExternalInput")
    pbT = nc.dram_tensor("pbT", [S, S], bf16, kind="ExternalInput")  # masked pos_bias^T
    wk = nc.dram_tensor("wk", [D, DH], bf16, kind="ExternalInput")  # Wk.T[:, h-cols]
    wv = nc.dram_tensor("wv", [D, DH], bf16, kind="ExternalInput")
    wq = nc.dram_tensor("wq", [D, DH], bf16, kind="ExternalInput")
    wo = nc.dram_tensor("wo", [DH, D], bf16, kind="ExternalInput")  # Wo.T[h-rows, :]
    out = nc.dram_tensor("out", [S, D], f32, kind="ExternalOutput")  # partial!

    with tile.TileContext(nc) as tc:
        with tc.tile_pool(name="persist", bufs=1) as persist:
            # resident across phases (per-partition KB in comments)
            ek_sb = [persist.tile([P, DH], bf16, name=f"ek{t}") for t in range(NT)]    # 16
            ekv_sb = [persist.tile([P, DH], bf16, name=f"ekv{t}") for t in range(NT)]  # 16
            eT_sb = [persist.tile([P, S], bf16, name=f"eT{t}") for t in range(NT - 1)] # 60
            sqT_sb = [persist.tile([P, S], bf16, name=f"sqT{t}") for t in range(4)]    # 16
            yT_sb = [persist.tile([P, S], bf16, name=f"yT{t}") for t in range(4)]      # 16
            ones_j = persist.tile([P, 1], bf16, name="ones_j")
            ones_row = persist.tile([1, 512], bf16, name="ones_row")
            stot_sb = persist.tile([1, 512], bf16, name="stot_sb")
            ktot_sb = persist.tile([1, 512], bf16, name="ktot_sb")
            nc.gpsimd.memset(ones_j[:], 1.0)
            nc.gpsimd.memset(ones_row[:], 1.0)
            # j-tile reach per 512-row i-block: j <= i-255
            CAPS = [2, 6, 10, 14]
            IB_MIN = [0 if j <= 2 else 1 if j <= 6 else 2 if j <= 10 else 3
                      for j in range(NT - 1)]

            # ---- phase A: k/v projection (all tokens, own d-half), exp ----
            # two token-halves so keyT/valT are only half-resident
            with (
                tc.tile_pool(name="pa", bufs=1) as pa,
                tc.tile_pool(name="pa_st", bufs=3) as pa_st,
                tc.tile_pool(name="ps_a", bufs=2, space="PSUM") as ps_a,
            ):
                pd = tc.alloc_tile_pool(name="pd", bufs=3)
                # weights as one [128, 8*512] tile: block kt at cols kt*512
                wk_sb = pa.tile([P, 8 * DH], bf16, name="wk_sb")
                wkv = wk[:, :].rearrange("(kt p) e -> p kt e", p=P)
                nc.sync.dma_start(out=wk_sb[:, 0:2048], in_=wkv[:, 0:4, :])
                nc.sync.dma_start(out=wk_sb[:, 2048:4096], in_=wkv[:, 4:8, :])
                wv_sb = pa.tile([P, 8 * DH], bf16, name="wv_sb")
                wvv = wv[:, :].rearrange("(kt p) e -> p kt e", p=P)
                nc.sync.dma_start(out=wv_sb[:, 0:2048], in_=wvv[:, 0:4, :])
                nc.sync.dma_start(out=wv_sb[:, 2048:4096], in_=wvv[:, 4:8, :])
                # token quarters, double-buffered so loads prefetch ahead of
                # the WAR release; phase-D tiles interleave into the DMA gaps
                for q in range(4):
                    cs = slice(q * 512, (q + 1) * 512)
                    keyT_sb = pa.tile(
                        [P, 8 * 512], bf16, tag="keyT_q", name="keyT_q", bufs=2
                    )
                    kv_ = keyT[:, :].rearrange("(kt p) s -> p kt s", p=P)
                    nc.sync.dma_start(out=keyT_sb[:, 0:2048], in_=kv_[:, 0:4, cs])
                    nc.sync.dma_start(out=keyT_sb[:, 2048:4096], in_=kv_[:, 4:8, cs])
                    valT_sb = pa.tile(
                        [P, 8 * 512], bf16, tag="valT_q", name="valT_q", bufs=2
                    )
                    vv_ = valT[:, :].rearrange("(kt p) s -> p kt s", p=P)
                    nc.sync.dma_start(out=valT_sb[:, 0:2048], in_=vv_[:, 0:4, cs])
                    nc.sync.dma_start(out=valT_sb[:, 2048:4096], in_=vv_[:, 4:8, cs])
                    for jt in range(4 * q, min(4 * q + 4, NT - 1)):
                        # only the unmasked column range is ever read
                        c0 = 512 * IB_MIN[jt]
                        pbt = pd.tile([P, S], bf16, tag="pbt")
                        nc.sync.dma_start(
                            out=pbt[:, c0:S], in_=pbT[jt * P : (jt + 1) * P, c0:S]
                        )
                        nc.scalar.activation(eT_sb[jt][:, c0:S], pbt[:, c0:S], AF.Exp)
                        nc.vector.tensor_scalar_add(
                            eT_sb[jt][:, c0:S], eT_sb[jt][:, c0:S], -1.0
                        )
                    for tl in range(4):
                        tt = q * 4 + tl
                        psk = ps_a.tile([P, DH], f32, tag="psk")
                        psv = ps_a.tile([P, DH], f32, tag="psv")
                        for kt in range(8):
                            c = kt * 512 + tl * P
                            nc.tensor.matmul(
                                psk[:],
                                keyT_sb[:, c : c + P],
                                wk_sb[:, kt * DH : (kt + 1) * DH],
                                start=(kt == 0),
                                stop=(kt == 7),
                            )
                        for kt in range(8):
                            c = kt * 512 + tl * P
                            nc.tensor.matmul(
                                psv[:],
                                valT_sb[:, c : c + P],
                                wv_sb[:, kt * DH : (kt + 1) * DH],
                                start=(kt == 0),
                                stop=(kt == 7),
                            )
                        ekf = pa_st.tile([P, DH], f32, tag="ekf")
                        nc.scalar.activation(ekf[:], psk[:], AF.Exp)
                        nc.vector.tensor_copy(ek_sb[tt][:], ekf[:])
                        nc.vector.tensor_mul(ekv_sb[tt][:], ekf[:], psv[:])
                pd.release()

            # ---- phase C: q^T projection + sigmoid ([e,i] orientation) ----
            with (
                tc.tile_pool(name="pc", bufs=1) as pc,
                tc.tile_pool(name="ps_c", bufs=2, space="PSUM") as ps_c,
            ):
                wq_sb = pc.tile([P, 8 * DH], bf16, name="wq_sb")
                wqv = wq[:, :].rearrange("(kt p) e -> p kt e", p=P)
                nc.sync.dma_start(out=wq_sb[:, 0:2048], in_=wqv[:, 0:4, :])
                nc.sync.dma_start(out=wq_sb[:, 2048:4096], in_=wqv[:, 4:8, :])
                for ib in range(4):
                    cs = slice(ib * 512, (ib + 1) * 512)
                    qT_sb = pc.tile(
                        [P, 8 * 512], bf16, tag="qT_q", name="qT_q", bufs=2
                    )
                    qv = queryT[:, :].rearrange("(kt p) s -> p kt s", p=P)
                    nc.sync.dma_start(out=qT_sb[:, 0:2048], in_=qv[:, 0:4, cs])
                    nc.sync.dma_start(out=qT_sb[:, 2048:4096], in_=qv[:, 4:8, cs])
                    for et in range(4):
                        psq = ps_c.tile([P, 512], f32, tag="psq")
                        for kt in range(8):
                            nc.tensor.matmul(
                                psq[:],
                                wq_sb[:, kt * DH + et * P : kt * DH + (et + 1) * P],
                                qT_sb[:, kt * 512 : (kt + 1) * 512],
                                start=(kt == 0),
                                stop=(kt == 7),
                            )
                        nc.scalar.activation(
                            sqT_sb[et][:, ib * 512 : (ib + 1) * 512], psq[:], AF.Sigmoid
                        )
                # Stot/Ktot: token-tile accumulation on the idle GpSimd,
                # then one M=1 matmul each for the partition reduction
                ps_s = tc.alloc_tile_pool(name="ps_s", bufs=1, space="PSUM")
                pacc = tc.alloc_tile_pool(name="pacc", bufs=1)
                sacc = pacc.tile([P, 512], f32, name="sacc")
                kacc = pacc.tile([P, 512], f32, name="kacc")
                nc.gpsimd.tensor_copy(sacc[:], ekv_sb[0][:])
                nc.gpsimd.tensor_copy(kacc[:], ek_sb[0][:])
                for jt in range(1, NT):
                    nc.gpsimd.tensor_add(sacc[:], sacc[:], ekv_sb[jt][:])
                    nc.gpsimd.tensor_add(kacc[:], kacc[:], ek_sb[jt][:])
                stp = ps_s.tile([1, 512], f32, tag="stp")
                ktp = ps_s.tile([1, 512], f32, tag="ktp")
                onesf = pacc.tile([P, 1], f32, name="onesf")
                nc.vector.memset(onesf[:], 1.0)
                nc.tensor.matmul(stp[:], onesf[:], sacc[:], start=True, stop=True)
                nc.tensor.matmul(ktp[:], onesf[:], kacc[:], start=True, stop=True)
                nc.vector.tensor_copy(stot_sb[:], stp[:])
                nc.vector.tensor_copy(ktot_sb[:], ktp[:])
                ps_s.release()
                pacc.release()

            # ---- phase E: num^T/den^T chains in [d,i]; y^T epilogue ----
            # descending cap order: long chains first (PE stays warm) and the
            # late i-blocks' y^T land early so phase F can overlap the tail
            with (
                tc.tile_pool(name="pe_ep", bufs=2) as pe_ep,
                tc.tile_pool(name="ps_e", bufs=2, space="PSUM") as ps_e,
            ):
                for ib in (3, 2, 1, 0):
                    csl = slice(ib * 512, (ib + 1) * 512)
                    cap = CAPS[ib]
                    for dt in range(4):
                        dsl = slice(dt * P, (dt + 1) * P)
                        na = ps_e.tile([P, 512], f32, tag="na")
                        da = ps_e.tile([P, 512], f32, tag="da")
                        # rank-1 dense term (all-j sum), then the expm1
                        # triangular correction over the reachable j-tiles
                        nc.tensor.matmul(
                            na[:], stot_sb[0:1, dt * P : (dt + 1) * P],
                            ones_row[:], start=True, stop=False,
                        )
                        for jt in range(cap + 1):
                            nc.tensor.matmul(
                                na[:],
                                ekv_sb[jt][:, dsl],
                                eT_sb[jt][:, csl],
                                start=False,
                                stop=(jt == cap),
                            )
                        nc.tensor.matmul(
                            da[:], ktot_sb[0:1, dt * P : (dt + 1) * P],
                            ones_row[:], start=True, stop=False,
                        )
                        for jt in range(cap + 1):
                            nc.tensor.matmul(
                                da[:],
                                ek_sb[jt][:, dsl],
                                eT_sb[jt][:, csl],
                                start=False,
                                stop=(jt == cap),
                            )
                        rec = pe_ep.tile([P, 512], f32, tag="rec")
                        nc.vector.reciprocal(rec[:], da[:])
                        tmp = pe_ep.tile([P, 512], f32, tag="tmp")
                        nc.vector.tensor_mul(tmp[:], na[:], rec[:])
                        # final gate-mul on the otherwise-idle GpSimd (SBUF-only)
                        nc.gpsimd.tensor_mul(
                            yT_sb[dt][:, csl], tmp[:], sqT_sb[dt][:, csl]
                        )

            # ---- phase F: partial output projection (y^T is the lhsT) ----
            with (
                tc.tile_pool(name="pf_o", bufs=3) as pf_o,
                tc.tile_pool(name="ps_fo", bufs=2, space="PSUM") as ps_fo,
            ):
                wo_sb = pf_o.tile([P, 4 * D], bf16, name="wo_sb", tag="wo_sb", bufs=1)
                wov = wo[:, :].rearrange("(dt p) e -> p dt e", p=P)
                nc.sync.dma_start(out=wo_sb[:, 0:2048], in_=wov[:, 0:2, :])
                nc.sync.dma_start(out=wo_sb[:, 2048:4096], in_=wov[:, 2:4, :])
                for it in range(NT - 1, -1, -1):
                    for es in range(2):
                        pso = ps_fo.tile([P, 512], f32, tag=f"pso{es}")
                        for dt in range(4):
                            nc.tensor.matmul(
                                pso[:],
                                yT_sb[dt][:, it * P : (it + 1) * P],
                                wo_sb[:, dt * D + es * 512 : dt * D + (es + 1) * 512],
                                start=(dt == 0),
                                stop=(dt == 3),
                            )
                        osb = pf_o.tile([P, 512], f32, tag="osb")
                        nc.scalar.activation(osb[:], pso[:], AF.Copy)
                        nc.sync.dma_start(
                            out=out[it * P : (it + 1) * P, es * 512 : (es + 1) * 512],
                            in_=osb[:],
                        )

    nc.compile()
    return nc


def _prep_inputs(inputs):
    bf = ml_dtypes.bfloat16
    query, key_in, value = inputs["query"], inputs["key_in"], inputs["value"]
    pos_bias = inputs["pos_bias"]

    jj = np.arange(S)
    pbT = pos_bias.T.copy()  # [j, i]
    pbT[~(jj[:, None] <= jj[None, :] - (W - 1))] = 0.0
    pbT = pbT.astype(bf)

    wq_t = np.ascontiguousarray(inputs["Wq"].T).astype(bf)  # [din, e]
    wk_t = np.ascontiguousarray(inputs["Wk"].T).astype(bf)
    wv_t = np.ascontiguousarray(inputs["Wv"].T).astype(bf)
    wo_t = np.ascontiguousarray(inputs["Wo"].T).astype(bf)  # [d, e']

    keyT_b = [np.ascontiguousarray(key_in[:, b, :].T).astype(bf) for b in range(B)]
    valT_b = [np.ascontiguousarray(value[:, b, :].T).astype(bf) for b in range(B)]
    qT_b = [np.ascontiguousarray(query[:, b, :].T).astype(bf) for b in range(B)]

    in_maps = []
    for c in range(NC):
        b, h = c // 2, c % 2
        hs = slice(h * DH, (h + 1) * DH)
        in_maps.append(
            {
                "keyT": keyT_b[b],
                "valT": valT_b[b],
                "queryT": qT_b[b],
                "pbT": pbT,
                "wk": np.ascontiguousarray(wk_t[:, hs]),
                "wv": np.ascontiguousarray(wv_t[:, hs]),
                "wq": np.ascontiguousarray(wq_t[:, hs]),
                "wo": np.ascontiguousarray(wo_t[hs, :]),
            }
        )
    return in_maps


def _run(inputs, trace=False):
    from concourse.bass_utils import run_bass_kernel_spmd

    if "nc" not in _CACHE:
        _CACHE["nc"] = _build()
    nc = _CACHE["nc"]

    in_maps = _prep_inputs(inputs)
    res = run_bass_kernel_spmd(nc, in_maps, core_ids=list(range(NC)), trace=trace)

    # unshard: partial sums over d-halves per batch
    full = np.empty((S, B, D), np.float32)
    for b in range(B):
        p0 = np.asarray(res.results[2 * b]["out"], np.float32)
        p1 = np.asarray(res.results[2 * b + 1]["out"], np.float32)
        full[:, b, :] = p0 + p1
    return full, res


def kernel(**inputs):
    inputs = {k: np.asarray(v) for k, v in inputs.items()}
    full, _ = _run(inputs, trace=False)
    return full


if __name__ == "__main__":
    inputs = np.load("/tmp/inputs.npy", allow_pickle=True).item()
    out = kernel(**inputs)
    print("out", out.shape, out.dtype)


# revision 29
# speedup vs baseline: 1.1270x; 1.0262x over previous
"""AFT-Local distributed Trainium2 kernel (8 NeuronCores).

Math (reference, with cancellations):
  q = query @ Wq.T; k = key_in @ Wk.T; v = value @ Wv.T      [S,B,D]
  E[i,j] = exp(pos_bias[i,j] * (j <= i-255))                 [S,S]
  num[i,b,:] = sum_j E[i,j] * (exp(k)*v)[j,b,:]
  den[i,b,:] = sum_j E[i,j] *  exp(k)[j,b,:]
  out = (sigmoid(q) * num / den) @ Wo.T
The max-subtractions in the reference cancel in num/den; all values are small
enough that plain exp is safe.

Distribution (v4+): pure data/tensor-parallel, ZERO device collectives (the
collective control path on this fleet has a ~90us fixed cost, impossible to
hide). Core c owns (batch b = c//2, d-half h = c%2): it projects k/v/q for
all 2048 tokens restricted to its 512 d-columns (no duplicated FLOPs), runs
the full [2048x2048] E-weighted attention on its slice entirely out of SBUF,
and computes a PARTIAL output projection over its d-half. The host sums each
core-pair's f32 partials while unsharding - the only cross-core data motion
in the whole scheme.

Kernel structure (v5): all matmuls bf16 with f32 PSUM accumulation, in long
accumulation chains into a single PSUM bank (keeps the PE HAM-warm). The
attention num/den and the q projection run in the TRANSPOSED [d,i]
orientation so y comes out as y^T and feeds the output projection directly -
no on-chip transposes anywhere. The local mask is pre-applied to pos_bias^T
on the host (static index mask) so the device only exponentiates.
"""

import os
import sys

import numpy as np
import ml_dtypes

sys.path.insert(0, "/opt/trn_rl_repo")

S, B, D, W = 2048, 4, 1024, 256
NC = 8
P = 128
NT = S // P  # 16 token/row tiles
DH = 512  # d-half owned per core

_CACHE = {}


def _build():
    import concourse.bass as bass
    import concourse.bacc as bacc
    import concourse.mybir as mybir
    import concourse.tile as tile

    f32 = mybir.dt.float32
    bf16 = mybir.dt.bfloat16
    AF = mybir.ActivationFunctionType

    nc = bacc.Bacc("TRN2", target_bir_lowering=False, debug=False, num_devices=NC)

    # per-core inputs (b = batch owned, h = d-half owned)
    keyT = nc.dram_tensor("keyT", [D, S], bf16, kind="ExternalInput")  # key_in[:,b,:].T
    valT = nc.dram_tensor("valT", [D, S], bf16, kind="ExternalInput")
    queryT = nc.dram_tensor("queryT", [D, S], bf16, kind="ExternalInput")
    pbT = nc.dram_tensor("pbT", [S, S], bf16, kind="ExternalInput")  # masked pos_bias^T
    wk = nc.dram_tensor("wk", [D, DH], bf16, kind="ExternalInput")  # Wk.T[:, h-cols]
    wv = nc.dram_tensor("wv", [D, DH], bf16, kind="ExternalInput")
    wq = nc.dram_tensor("wq", [D, DH], bf16, kind="ExternalInput")
    wo = nc.dram_tensor("wo", [DH, D], bf16, kind="ExternalInput")  # Wo.T[h-rows, :]
    out = nc.dram_tensor("out", [S, D], f32, kind="ExternalOutput")  # partial!

    with tile.TileContext(nc) as tc:
        with tc.tile_pool(name="persist", bufs=1) as persist:
            # resident across phases (per-partition KB in comments)
            ek_sb = [persist.tile([P, DH], bf16, name=f"ek{t}") for t in range(NT)]    # 16
            ekv_sb = [persist.tile([P, DH], bf16, name=f"ekv{t}") for t in range(NT)]  # 16
            eT_sb = [persist.tile([P, S], bf16, name=f"eT{t}") for t in range(NT - 1)] # 60
            sqT_sb = [persist.tile([P, S], bf16, name=f"sqT{t}") for t in range(4)]    # 16
            yT_sb = [persist.tile([P, S], bf16, name=f"yT{t}") for t in range(4)]      # 16
            ones_j = persist.tile([P, 1], bf16, name="ones_j")
            ones_row = persist.tile([1, 512], bf16, name="ones_row")
            stot_sb = persist.tile([1, 512], bf16, name="stot_sb")
            ktot_sb = persist.tile([1, 512], bf16, name="ktot_sb")
            nc.gpsimd.memset(ones_j[:], 1.0)
            nc.gpsimd.memset(ones_row[:], 1.0)
            # j-tile reach per 256-row i-block ib: jt <= 2*ib  (j <= i-255)
            IB_MIN = [(j + 1) // 2 for j in range(NT - 1)]

            # ---- phase A: k/v projection (all tokens, own d-half), exp ----
            # two token-halves so keyT/valT are only half-resident
            with (
                tc.tile_pool(name="pa", bufs=1) as pa,
                tc.tile_pool(name="pa_st", bufs=3) as pa_st,
                tc.tile_pool(name="ps_a", bufs=2, space="PSUM") as ps_a,
            ):
                pd = tc.alloc_tile_pool(name="pd", bufs=3)
                # weights as one [128, 8*512] tile: block kt at cols kt*512
                wk_sb = pa.tile([P, 8 * DH], bf16, name="wk_sb")
                wkv = wk[:, :].rearrange("(kt p) e -> p kt e", p=P)
                nc.sync.dma_start(out=wk_sb[:, 0:2048], in_=wkv[:, 0:4, :])
                nc.sync.dma_start(out=wk_sb[:, 2048:4096], in_=wkv[:, 4:8, :])
                wv_sb = pa.tile([P, 8 * DH], bf16, name="wv_sb")
                wvv = wv[:, :].rearrange("(kt p) e -> p kt e", p=P)
                nc.sync.dma_start(out=wv_sb[:, 0:2048], in_=wvv[:, 0:4, :])
                nc.sync.dma_start(out=wv_sb[:, 2048:4096], in_=wvv[:, 4:8, :])
                # token quarters, double-buffered so loads prefetch ahead of
                # the WAR release; phase-D tiles interleave into the DMA gaps
                for q in range(4):
                    cs = slice(q * 512, (q + 1) * 512)
                    keyT_sb = pa.tile(
                        [P, 8 * 512], bf16, tag="keyT_q", name="keyT_q", bufs=2
                    )
                    kv_ = keyT[:, :].rearrange("(kt p) s -> p kt s", p=P)
                    nc.sync.dma_start(out=keyT_sb[:, 0:2048], in_=kv_[:, 0:4, cs])
                    nc.sync.dma_start(out=keyT_sb[:, 2048:4096], in_=kv_[:, 4:8, cs])
                    valT_sb = pa.tile(
                        [P, 8 * 512], bf16, tag="valT_q", name="valT_q", bufs=2
                    )
                    vv_ = valT[:, :].rearrange("(kt p) s -> p kt s", p=P)
                    nc.sync.dma_start(out=valT_sb[:, 0:2048], in_=vv_[:, 0:4, cs])
                    nc.sync.dma_start(out=valT_sb[:, 2048:4096], in_=vv_[:, 4:8, cs])
                    for jt in range(4 * q, min(4 * q + 4, NT - 1)):
                        # only the unmasked column range is ever read
                        c0 = 256 * IB_MIN[jt]
                        pbt = pd.tile([P, S], bf16, tag="pbt")
                        nc.sync.dma_start(
                            out=pbt[:, c0:S], in_=pbT[jt * P : (jt + 1) * P, c0:S]
                        )
                        nc.scalar.activation(eT_sb[jt][:, c0:S], pbt[:, c0:S], AF.Exp)
                        nc.vector.tensor_scalar_add(
                            eT_sb[jt][:, c0:S], eT_sb[jt][:, c0:S], -1.0
                        )
                    for tl in range(4):
                        tt = q * 4 + tl
                        psk = ps_a.tile([P, DH], f32, tag="psk")
                        psv = ps_a.tile([P, DH], f32, tag="psv")
                        for kt in range(8):
                            c = kt * 512 + tl * P
                            nc.tensor.matmul(
                                psk[:],
                                keyT_sb[:, c : c + P],
                                wk_sb[:, kt * DH : (kt + 1) * DH],
                                start=(kt == 0),
                                stop=(kt == 7),
                            )
                        for kt in range(8):
                            c = kt * 512 + tl * P
                            nc.tensor.matmul(
                                psv[:],
                                valT_sb[:, c : c + P],
                                wv_sb[:, kt * DH : (kt + 1) * DH],
                                start=(kt == 0),
                                stop=(kt == 7),
                            )
                        ekf = pa_st.tile([P, DH], f32, tag="ekf")
                        nc.scalar.activation(ekf[:], psk[:], AF.Exp)
                        nc.vector.tensor_copy(ek_sb[tt][:], ekf[:])
                        nc.vector.tensor_mul(ekv_sb[tt][:], ekf[:], psv[:])
                pd.release()

            # ---- phase C: q^T projection + sigmoid ([e,i] orientation) ----
            with (
                tc.tile_pool(name="pc", bufs=1) as pc,
                tc.tile_pool(name="ps_c", bufs=2, space="PSUM") as ps_c,
            ):
                wq_sb = pc.tile([P, 8 * DH], bf16, name="wq_sb")
                wqv = wq[:, :].rearrange("(kt p) e -> p kt e", p=P)
                nc.sync.dma_start(out=wq_sb[:, 0:2048], in_=wqv[:, 0:4, :])
                nc.sync.dma_start(out=wq_sb[:, 2048:4096], in_=wqv[:, 4:8, :])
                for ib in range(4):
                    cs = slice(ib * 512, (ib + 1) * 512)
                    qT_sb = pc.tile(
                        [P, 8 * 512], bf16, tag="qT_q", name="qT_q", bufs=2
                    )
                    qv = queryT[:, :].rearrange("(kt p) s -> p kt s", p=P)
                    nc.sync.dma_start(out=qT_sb[:, 0:2048], in_=qv[:, 0:4, cs])
                    nc.sync.dma_start(out=qT_sb[:, 2048:4096], in_=qv[:, 4:8, cs])
                    for et in range(4):
                        psq = ps_c.tile([P, 512], f32, tag="psq")
                        for kt in range(8):
                            nc.tensor.matmul(
                                psq[:],
                                wq_sb[:, kt * DH + et * P : kt * DH + (et + 1) * P],
                                qT_sb[:, kt * 512 : (kt + 1) * 512],
                                start=(kt == 0),
                                stop=(kt == 7),
                            )
                        nc.scalar.activation(
                            sqT_sb[et][:, ib * 512 : (ib + 1) * 512], psq[:], AF.Sigmoid
                        )
                # Stot/Ktot: token-tile accumulation on the idle GpSimd,
                # then one M=1 matmul each for the partition reduction
                ps_s = tc.alloc_tile_pool(name="ps_s", bufs=1, space="PSUM")
                pacc = tc.alloc_tile_pool(name="pacc", bufs=1)
                sacc = pacc.tile([P, 512], f32, name="sacc")
                kacc = pacc.tile([P, 512], f32, name="kacc")
                nc.gpsimd.tensor_copy(sacc[:], ekv_sb[0][:])
                nc.gpsimd.tensor_copy(kacc[:], ek_sb[0][:])
                for jt in range(1, NT):
                    nc.gpsimd.tensor_add(sacc[:], sacc[:], ekv_sb[jt][:])
                    nc.gpsimd.tensor_add(kacc[:], kacc[:], ek_sb[jt][:])
                stp = ps_s.tile([1, 512], f32, tag="stp")
                ktp = ps_s.tile([1, 512], f32, tag="ktp")
                onesf = pacc.tile([P, 1], f32, name="onesf")
                nc.vector.memset(onesf[:], 1.0)
                nc.tensor.matmul(stp[:], onesf[:], sacc[:], start=True, stop=True)
                nc.tensor.matmul(ktp[:], onesf[:], kacc[:], start=True, stop=True)
                nc.vector.tensor_copy(stot_sb[:], stp[:])
                nc.vector.tensor_copy(ktot_sb[:], ktp[:])
                ps_s.release()
                pacc.release()

            # ---- phases E+F fused: num^T/den^T triangular chains in [d,i],
            # y^T epilogue, and the partial output projection interleaved so
            # o-proj chains fill the short-chain bubbles. Descending ib keeps
            # the PE on long chains first (HAM-warm) and releases late
            # i-blocks early for the o-proj.
            with (
                tc.tile_pool(name="pe_ep", bufs=3) as pe_ep,
                tc.tile_pool(name="pf_o", bufs=3) as pf_o,
                tc.tile_pool(name="ps_e", bufs=2, space="PSUM") as ps_e,
                tc.tile_pool(name="ps_fo", bufs=2, space="PSUM") as ps_fo,
            ):
                wo_sb = pf_o.tile([P, 4 * D], bf16, name="wo_sb", tag="wo_sb", bufs=1)
                wov = wo[:, :].rearrange("(dt p) e -> p dt e", p=P)
                nc.sync.dma_start(out=wo_sb[:, 0:2048], in_=wov[:, 0:2, :])
                nc.sync.dma_start(out=wo_sb[:, 2048:4096], in_=wov[:, 2:4, :])
                for ib in range(7, -1, -1):
                    csl = slice(ib * 256, (ib + 1) * 256)
                    cap = 2 * ib
                    for dt in range(4):
                        dsl = slice(dt * P, (dt + 1) * P)
                        na = ps_e.tile([P, 256], f32, tag="na")
                        da = ps_e.tile([P, 256], f32, tag="da")
                        for jt in range(cap + 1):
                            nc.tensor.matmul(
                                na[:],
                                ekv_sb[jt][:, dsl],
                                eT_sb[jt][:, csl],
                                start=(jt == 0),
                                stop=False,
                            )
                        nc.tensor.matmul(
                            na[:], stot_sb[0:1, dt * P : (dt + 1) * P],
                            ones_row[0:1, 0:256], start=False, stop=True,
                        )
                        for jt in range(cap + 1):
                            nc.tensor.matmul(
                                da[:],
                                ek_sb[jt][:, dsl],
                                eT_sb[jt][:, csl],
                                start=(jt == 0),
                                stop=False,
                            )
                        nc.tensor.matmul(
                            da[:], ktot_sb[0:1, dt * P : (dt + 1) * P],
                            ones_row[0:1, 0:256], start=False, stop=True,
                        )
                        rec = pe_ep.tile([P, 256], f32, tag="rec")
                        nc.vector.reciprocal(rec[:], da[:])
                        tmp = pe_ep.tile([P, 256], f32, tag="tmp")
                        nc.vector.tensor_mul(tmp[:], na[:], rec[:])
                        # final gate-mul on the otherwise-idle GpSimd (SBUF-only)
                        nc.gpsimd.tensor_mul(
                            yT_sb[dt][:, csl], tmp[:], sqT_sb[dt][:, csl]
                        )
                    for it in (2 * ib + 1, 2 * ib):
                        for es in range(2):
                            pso = ps_fo.tile([P, 512], f32, tag=f"pso{es}")
                            for dt in range(4):
                                nc.tensor.matmul(
                                    pso[:],
                                    yT_sb[dt][:, it * P : (it + 1) * P],
                                    wo_sb[:, dt * D + es * 512 : dt * D + (es + 1) * 512],
                                    start=(dt == 0),
                                    stop=(dt == 3),
                                )
                            osb = pf_o.tile([P, 512], f32, tag="osb")
                            nc.scalar.activation(osb[:], pso[:], AF.Copy)
                            nc.sync.dma_start(
                                out=out[it * P : (it + 1) * P, es * 512 : (es + 1) * 512],
                                in_=osb[:],
                            )

    nc.compile()
    return nc


def _prep_inputs(inputs):
    bf = ml_dtypes.bfloat16
    query, key_in, value = inputs["query"], inputs["key_in"], inputs["value"]
    pos_bias = inputs["pos_bias"]

    jj = np.arange(S)
    pbT = pos_bias.T.copy()  # [j, i]
    pbT[~(jj[:, None] <= jj[None, :] - (W - 1))] = 0.0
    pbT = pbT.astype(bf)

    wq_t = np.ascontiguousarray(inputs["Wq"].T).astype(bf)  # [din, e]
    wk_t = np.ascontiguousarray(inputs["Wk"].T).astype(bf)
    wv_t = np.ascontiguousarray(inputs["Wv"].T).astype(bf)
    wo_t = np.ascontiguousarray(inputs["Wo"].T).astype(bf)  # [d, e']

    keyT_b = [np.ascontiguousarray(key_in[:, b, :].T).astype(bf) for b in range(B)]
    valT_b = [np.ascontiguousarray(value[:, b, :].T).astype(bf) for b in range(B)]
    qT_b = [np.ascontiguousarray(query[:, b, :].T).astype(bf) for b in range(B)]

    in_maps = []
    for c in range(NC):
        b, h = c // 2, c % 2
        hs = slice(h * DH, (h + 1) * DH)
        in_maps.append(
            {
                "keyT": keyT_b[b],
                "valT": valT_b[b],
                "queryT": qT_b[b],
                "pbT": pbT,
                "wk": np.ascontiguousarray(wk_t[:, hs]),
                "wv": np.ascontiguousarray(wv_t[:, hs]),
                "wq": np.ascontiguousarray(wq_t[:, hs]),
                "wo": np.ascontiguousarray(wo_t[hs, :]),
            }
        )
    return in_maps


def _run(inputs, trace=False):
    from concourse.bass_utils import run_bass_kernel_spmd

    if "nc" not in _CACHE:
        _CACHE["nc"] = _build()
    nc = _CACHE["nc"]

    in_maps = _prep_inputs(inputs)
    res = run_bass_kernel_spmd(nc, in_maps, core_ids=list(range(NC)), trace=trace)

    # unshard: partial sums over d-halves per batch
    full = np.empty((S, B, D), np.float32)
    for b in range(B):
        p0 = np.asarray(res.results[2 * b]["out"], np.float32)
        p1 = np.asarray(res.results[2 * b + 1]["out"], np.float32)
        full[:, b, :] = p0 + p1
    return full, res


def kernel(**inputs):
    inputs = {k: np.asarray(v) for k, v in inputs.items()}
    full, _ = _run(inputs, trace=False)
    return full


if __name__ == "__main__":
    inputs = np.load("/tmp/inputs.npy", allow_pickle=True).item()
    out = kernel(**inputs)
    print("out", out.shape, out.dtype)


# revision 31
# speedup vs baseline: 1.1608x; 1.0300x over previous
"""AFT-Local distributed Trainium2 kernel (8 NeuronCores).

Math (reference, with cancellations):
  q = query @ Wq.T; k = key_in @ Wk.T; v = value @ Wv.T      [S,B,D]
  E[i,j] = exp(pos_bias[i,j] * (j <= i-255))                 [S,S]
  num[i,b,:] = sum_j E[i,j] * (exp(k)*v)[j,b,:]
  den[i,b,:] = sum_j E[i,j] *  exp(k)[j,b,:]
  out = (sigmoid(q) * num / den) @ Wo.T
The max-subtractions in the reference cancel in num/den; all values are small
enough that plain exp is safe.

Distribution (v4+): pure data/tensor-parallel, ZERO device collectives (the
collective control path on this fleet has a ~90us fixed cost, impossible to
hide). Core c owns (batch b = c//2, d-half h = c%2): it projects k/v/q for
all 2048 tokens restricted to its 512 d-columns (no duplicated FLOPs), runs
the full [2048x2048] E-weighted attention on its slice entirely out of SBUF,
and computes a PARTIAL output projection over its d-half. The host sums each
core-pair's f32 partials while unsharding - the only cross-core data motion
in the whole scheme.

Kernel structure (v5): all matmuls bf16 with f32 PSUM accumulation, in long
accumulation chains into a single PSUM bank (keeps the PE HAM-warm). The
attention num/den and the q projection run in the TRANSPOSED [d,i]
orientation so y comes out as y^T and feeds the output projection directly -
no on-chip transposes anywhere. The local mask is pre-applied to pos_bias^T
on the host (static index mask) so the device only exponentiates.
"""

import os
import sys

import numpy as np
import ml_dtypes

sys.path.insert(0, "/opt/trn_rl_repo")

S, B, D, W = 2048, 4, 1024, 256
NC = 8
P = 128
NT = S // P  # 16 token/row tiles
DH = 512  # d-half owned per core

_CACHE = {}


def _build():
    import concourse.bass as bass
    import concourse.bacc as bacc
    import concourse.mybir as mybir
    import concourse.tile as tile

    f32 = mybir.dt.float32
    bf16 = mybir.dt.bfloat16
    AF = mybir.ActivationFunctionType

    nc = bacc.Bacc("TRN2", target_bir_lowering=False, debug=False, num_devices=NC)

    # per-core inputs (b = batch owned, h = d-half owned)
    keyT = nc.dram_tensor("keyT", [D, S], bf16, kind="ExternalInput")  # key_in[:,b,:].T
    valT = nc.dram_tensor("valT", [D, S], bf16, kind="ExternalInput")
    queryT = nc.dram_tensor("queryT", [D, S], bf16, kind="ExternalInput")
    pbT = nc.dram_tensor("pbT", [S, S], bf16, kind="ExternalInput")  # masked pos_bias^T
    wk = nc.dram_tensor("wk", [D, DH], bf16, kind="ExternalInput")  # Wk.T[:, h-cols]
    wv = nc.dram_tensor("wv", [D, DH], bf16, kind="ExternalInput")
    wq = nc.dram_tensor("wq", [D, DH], bf16, kind="ExternalInput")
    wo = nc.dram_tensor("wo", [DH, D], bf16, kind="ExternalInput")  # Wo.T[h-rows, :]
    out = nc.dram_tensor("out", [S, D], f32, kind="ExternalOutput")  # partial!

    with tile.TileContext(nc) as tc:
        with tc.tile_pool(name="persist", bufs=1) as persist:
            # resident across phases (per-partition KB in comments)
            # j-tile reach per 256-row i-block ib: jt <= 2*ib  (j <= i-255)
            IB_MIN = [(j + 1) // 2 for j in range(NT - 1)]
            ek_sb = [persist.tile([P, DH], bf16, name=f"ek{t}") for t in range(NT)]    # 16
            ekv_sb = [persist.tile([P, DH], bf16, name=f"ekv{t}") for t in range(NT)]  # 16
            # eT tiles hold only the unmasked column range [256*IB_MIN[jt], S)
            eT_sb = [
                persist.tile([P, S - 256 * IB_MIN[t]], bf16, name=f"eT{t}")
                for t in range(NT - 1)
            ]  # 32
            sqT_sb = [persist.tile([P, S], bf16, name=f"sqT{t}") for t in range(4)]    # 16
            yT_sb = [persist.tile([P, S], bf16, name=f"yT{t}") for t in range(4)]      # 16
            stot_sb = persist.tile([1, 512], f32, name="stot_sb")
            ktot_sb = persist.tile([1, 512], f32, name="ktot_sb")
            stotT_sb = persist.tile([P, 4], f32, name="stotT_sb")
            ktotT_sb = persist.tile([P, 4], f32, name="ktotT_sb")
            dscr = tc.alloc_tile_pool(name="dscr", bufs=1, space="DRAM")
            sd_dram = dscr.tile([1, 512], f32, name="sd_dram")
            kd_dram = dscr.tile([1, 512], f32, name="kd_dram")

            # ---- phase A: k/v projection (all tokens, own d-half), exp ----
            # two token-halves so keyT/valT are only half-resident
            with (
                tc.tile_pool(name="pa", bufs=1) as pa,
                tc.tile_pool(name="pa_st", bufs=3) as pa_st,
                tc.tile_pool(name="ps_a", bufs=2, space="PSUM") as ps_a,
            ):
                pd = tc.alloc_tile_pool(name="pd", bufs=3)
                # weights as one [128, 8*512] tile: block kt at cols kt*512
                wk_sb = pa.tile([P, 8 * DH], bf16, name="wk_sb")
                wkv = wk[:, :].rearrange("(kt p) e -> p kt e", p=P)
                nc.sync.dma_start(out=wk_sb[:, 0:2048], in_=wkv[:, 0:4, :])
                nc.sync.dma_start(out=wk_sb[:, 2048:4096], in_=wkv[:, 4:8, :])
                wv_sb = pa.tile([P, 8 * DH], bf16, name="wv_sb")
                wvv = wv[:, :].rearrange("(kt p) e -> p kt e", p=P)
                nc.sync.dma_start(out=wv_sb[:, 0:2048], in_=wvv[:, 0:4, :])
                nc.sync.dma_start(out=wv_sb[:, 2048:4096], in_=wvv[:, 4:8, :])
                # token quarters, double-buffered so loads prefetch ahead of
                # the WAR release; phase-D tiles interleave into the DMA gaps
                for q in range(4):
                    cs = slice(q * 512, (q + 1) * 512)
                    keyT_sb = pa.tile(
                        [P, 8 * 512], bf16, tag="keyT_q", name="keyT_q", bufs=2
                    )
                    kv_ = keyT[:, :].rearrange("(kt p) s -> p kt s", p=P)
                    nc.sync.dma_start(out=keyT_sb[:, 0:2048], in_=kv_[:, 0:4, cs])
                    nc.sync.dma_start(out=keyT_sb[:, 2048:4096], in_=kv_[:, 4:8, cs])
                    valT_sb = pa.tile(
                        [P, 8 * 512], bf16, tag="valT_q", name="valT_q", bufs=2
                    )
                    vv_ = valT[:, :].rearrange("(kt p) s -> p kt s", p=P)
                    nc.sync.dma_start(out=valT_sb[:, 0:2048], in_=vv_[:, 0:4, cs])
                    nc.sync.dma_start(out=valT_sb[:, 2048:4096], in_=vv_[:, 4:8, cs])
                    for jt in range(4 * q, min(4 * q + 4, NT - 1)):
                        # only the unmasked column range is ever read/stored
                        c0 = 256 * IB_MIN[jt]
                        n = S - c0
                        pbt = pd.tile([P, S], bf16, tag="pbt")
                        nc.sync.dma_start(
                            out=pbt[:, 0:n], in_=pbT[jt * P : (jt + 1) * P, c0:S]
                        )
                        nc.scalar.activation(eT_sb[jt][:, 0:n], pbt[:, 0:n], AF.Exp)
                        nc.vector.tensor_scalar_add(
                            eT_sb[jt][:, 0:n], eT_sb[jt][:, 0:n], -1.0
                        )
                    for tl in range(4):
                        tt = q * 4 + tl
                        psk = ps_a.tile([P, DH], f32, tag="psk")
                        psv = ps_a.tile([P, DH], f32, tag="psv")
                        for kt in range(8):
                            c = kt * 512 + tl * P
                            nc.tensor.matmul(
                                psk[:],
                                keyT_sb[:, c : c + P],
                                wk_sb[:, kt * DH : (kt + 1) * DH],
                                start=(kt == 0),
                                stop=(kt == 7),
                            )
                        for kt in range(8):
                            c = kt * 512 + tl * P
                            nc.tensor.matmul(
                                psv[:],
                                valT_sb[:, c : c + P],
                                wv_sb[:, kt * DH : (kt + 1) * DH],
                                start=(kt == 0),
                                stop=(kt == 7),
                            )
                        ekf = pa_st.tile([P, DH], f32, tag="ekf")
                        nc.scalar.activation(ekf[:], psk[:], AF.Exp)
                        nc.vector.tensor_copy(ek_sb[tt][:], ekf[:])
                        nc.vector.tensor_mul(ekv_sb[tt][:], ekf[:], psv[:])
                pd.release()

            # ---- phase C: q^T projection + sigmoid ([e,i] orientation) ----
            with (
                tc.tile_pool(name="pc", bufs=1) as pc,
                tc.tile_pool(name="ps_c", bufs=2, space="PSUM") as ps_c,
            ):
                wq_sb = pc.tile([P, 8 * DH], bf16, name="wq_sb")
                wqv = wq[:, :].rearrange("(kt p) e -> p kt e", p=P)
                nc.sync.dma_start(out=wq_sb[:, 0:2048], in_=wqv[:, 0:4, :])
                nc.sync.dma_start(out=wq_sb[:, 2048:4096], in_=wqv[:, 4:8, :])
                for ib in range(4):
                    cs = slice(ib * 512, (ib + 1) * 512)
                    qT_sb = pc.tile(
                        [P, 8 * 512], bf16, tag="qT_q", name="qT_q", bufs=2
                    )
                    qv = queryT[:, :].rearrange("(kt p) s -> p kt s", p=P)
                    nc.sync.dma_start(out=qT_sb[:, 0:2048], in_=qv[:, 0:4, cs])
                    nc.sync.dma_start(out=qT_sb[:, 2048:4096], in_=qv[:, 4:8, cs])
                    for et in range(4):
                        psq = ps_c.tile([P, 512], f32, tag="psq")
                        for kt in range(8):
                            nc.tensor.matmul(
                                psq[:],
                                wq_sb[:, kt * DH + et * P : kt * DH + (et + 1) * P],
                                qT_sb[:, kt * 512 : (kt + 1) * 512],
                                start=(kt == 0),
                                stop=(kt == 7),
                            )
                        nc.scalar.activation(
                            sqT_sb[et][:, ib * 512 : (ib + 1) * 512], psq[:], AF.Sigmoid
                        )
                # Stot/Ktot: token-tile accumulation on the idle GpSimd,
                # then one M=1 matmul each for the partition reduction
                ps_s = tc.alloc_tile_pool(name="ps_s", bufs=1, space="PSUM")
                pacc = tc.alloc_tile_pool(name="pacc", bufs=1)
                sacc = pacc.tile([P, 512], f32, name="sacc")
                kacc = pacc.tile([P, 512], f32, name="kacc")
                nc.gpsimd.tensor_copy(sacc[:], ekv_sb[0][:])
                nc.gpsimd.tensor_copy(kacc[:], ek_sb[0][:])
                for jt in range(1, NT):
                    nc.gpsimd.tensor_add(sacc[:], sacc[:], ekv_sb[jt][:])
                    nc.gpsimd.tensor_add(kacc[:], kacc[:], ek_sb[jt][:])
                stp = ps_s.tile([1, 512], f32, tag="stp")
                ktp = ps_s.tile([1, 512], f32, tag="ktp")
                onesf = pacc.tile([P, 1], f32, name="onesf")
                nc.vector.memset(onesf[:], 1.0)
                nc.tensor.matmul(stp[:], onesf[:], sacc[:], start=True, stop=True)
                nc.tensor.matmul(ktp[:], onesf[:], kacc[:], start=True, stop=True)
                nc.vector.tensor_copy(stot_sb[:], stp[:])
                nc.vector.tensor_copy(ktot_sb[:], ktp[:])
                ps_s.release()
                pacc.release()
                # relayout [1, 512] -> [128, 4] (d on partitions) via DRAM
                nc.sync.dma_start(out=sd_dram[:], in_=stot_sb[:])
                nc.sync.dma_start(out=kd_dram[:], in_=ktot_sb[:])
                nc.sync.dma_start(
                    out=stotT_sb[:],
                    in_=sd_dram[0:1, :].rearrange("o (dt p) -> (o p) dt", p=P),
                )
                nc.sync.dma_start(
                    out=ktotT_sb[:],
                    in_=kd_dram[0:1, :].rearrange("o (dt p) -> (o p) dt", p=P),
                )

            # ---- phases E+F fused: num^T/den^T triangular chains in [d,i],
            # y^T epilogue, and the partial output projection interleaved so
            # o-proj chains fill the short-chain bubbles. Descending ib keeps
            # the PE on long chains first (HAM-warm) and releases late
            # i-blocks early for the o-proj.
            with (
                tc.tile_pool(name="pe_ep", bufs=3) as pe_ep,
                tc.tile_pool(name="pf_o", bufs=3) as pf_o,
                tc.tile_pool(name="ps_e", bufs=2, space="PSUM") as ps_e,
                tc.tile_pool(name="ps_fo", bufs=2, space="PSUM") as ps_fo,
            ):
                wo_sb = pf_o.tile([P, 4 * D], bf16, name="wo_sb", tag="wo_sb", bufs=1)
                wov = wo[:, :].rearrange("(dt p) e -> p dt e", p=P)
                nc.sync.dma_start(out=wo_sb[:, 0:2048], in_=wov[:, 0:2, :])
                nc.sync.dma_start(out=wo_sb[:, 2048:4096], in_=wov[:, 2:4, :])
                for ib in range(7, -1, -1):
                    csl = slice(ib * 256, (ib + 1) * 256)
                    cap = 2 * ib
                    for dt in range(4):
                        dsl = slice(dt * P, (dt + 1) * P)
                        na = ps_e.tile([P, 256], f32, tag="na")
                        da = ps_e.tile([P, 256], f32, tag="da")
                        for jt in range(cap + 1):
                            c0 = 256 * IB_MIN[jt]
                            esl = slice(csl.start - c0, csl.stop - c0)
                            nc.tensor.matmul(
                                na[:],
                                ekv_sb[jt][:, dsl],
                                eT_sb[jt][:, esl],
                                start=(jt == 0),
                                stop=(jt == cap),
                            )
                        for jt in range(cap + 1):
                            c0 = 256 * IB_MIN[jt]
                            esl = slice(csl.start - c0, csl.stop - c0)
                            nc.tensor.matmul(
                                da[:],
                                ek_sb[jt][:, dsl],
                                eT_sb[jt][:, esl],
                                start=(jt == 0),
                                stop=(jt == cap),
                            )
                        # dense-term bias adds on ACT (per-partition = per-d)
                        dn = pe_ep.tile([P, 256], f32, tag="dn")
                        nc.scalar.activation(
                            dn[:], da[:], AF.Identity, bias=ktotT_sb[:, dt : dt + 1]
                        )
                        nm = pe_ep.tile([P, 256], f32, tag="nm")
                        nc.scalar.activation(
                            nm[:], na[:], AF.Identity, bias=stotT_sb[:, dt : dt + 1]
                        )
                        rec = pe_ep.tile([P, 256], f32, tag="rec")
                        nc.vector.reciprocal(rec[:], dn[:])
                        tmp = pe_ep.tile([P, 256], f32, tag="tmp")
                        nc.vector.tensor_mul(tmp[:], nm[:], rec[:])
                        # final gate-mul on the otherwise-idle GpSimd (SBUF-only)
                        nc.gpsimd.tensor_mul(
                            yT_sb[dt][:, csl], tmp[:], sqT_sb[dt][:, csl]
                        )
                    for it in (2 * ib + 1, 2 * ib):
                        for es in range(2):
                            pso = ps_fo.tile([P, 512], f32, tag=f"pso{es}")
                            for dt in range(4):
                                nc.tensor.matmul(
                                    pso[:],
                                    yT_sb[dt][:, it * P : (it + 1) * P],
                                    wo_sb[:, dt * D + es * 512 : dt * D + (es + 1) * 512],
                                    start=(dt == 0),
                                    stop=(dt == 3),
                                )
                            osb = pf_o.tile([P, 512], f32, tag="osb")
                            nc.scalar.activation(osb[:], pso[:], AF.Copy)
                            nc.sync.dma_start(
                                out=out[it * P : (it + 1) * P, es * 512 : (es + 1) * 512],
                                in_=osb[:],
                            )

    nc.compile()
    return nc


def _prep_inputs(inputs):
    bf = ml_dtypes.bfloat16
    query, key_in, value = inputs["query"], inputs["key_in"], inputs["value"]
    pos_bias = inputs["pos_bias"]

    jj = np.arange(S)
    pbT = pos_bias.T.copy()  # [j, i]
    pbT[~(jj[:, None] <= jj[None, :] - (W - 1))] = 0.0
    pbT = pbT.astype(bf)

    wq_t = np.ascontiguousarray(inputs["Wq"].T).astype(bf)  # [din, e]
    wk_t = np.ascontiguousarray(inputs["Wk"].T).astype(bf)
    wv_t = np.ascontiguousarray(inputs["Wv"].T).astype(bf)
    wo_t = np.ascontiguousarray(inputs["Wo"].T).astype(bf)  # [d, e']

    keyT_b = [np.ascontiguousarray(key_in[:, b, :].T).astype(bf) for b in range(B)]
    valT_b = [np.ascontiguousarray(value[:, b, :].T).astype(bf) for b in range(B)]
    qT_b = [np.ascontiguousarray(query[:, b, :].T).astype(bf) for b in range(B)]

    in_maps = []
    for c in range(NC):
        b, h = c // 2, c % 2
        hs = slice(h * DH, (h + 1) * DH)
        in_maps.append(
            {
                "keyT": keyT_b[b],
                "valT": valT_b[b],
                "queryT": qT_b[b],
                "pbT": pbT,
                "wk": np.ascontiguousarray(wk_t[:, hs]),
                "wv": np.ascontiguousarray(wv_t[:, hs]),
                "wq": np.ascontiguousarray(wq_t[:, hs]),
                "wo": np.ascontiguousarray(wo_t[hs, :]),
            }
        )
    return in_maps


def _run(inputs, trace=False):
    from concourse.bass_utils import run_bass_kernel_spmd

    if "nc" not in _CACHE:
        _CACHE["nc"] = _build()
    nc = _CACHE["nc"]

    in_maps = _prep_inputs(inputs)
    res = run_bass_kernel_spmd(nc, in_maps, core_ids=list(range(NC)), trace=trace)

    # unshard: partial sums over d-halves per batch
    full = np.empty((S, B, D), np.float32)
    for b in range(B):
        p0 = np.asarray(res.results[2 * b]["out"], np.float32)
        p1 = np.asarray(res.results[2 * b + 1]["out"], np.float32)
        full[:, b, :] = p0 + p1
    return full, res


def kernel(**inputs):
    inputs = {k: np.asarray(v) for k, v in inputs.items()}
    full, _ = _run(inputs, trace=False)
    return full


if __name__ == "__main__":
    inputs = np.load("/tmp/inputs.npy", allow_pickle=True).item()
    out = kernel(**inputs)
    print("out", out.shape, out.dtype)
